# revision 10
# baseline (speedup 1.0000x reference)
"""Trainium2 Bass kernel for nn_EnhancedMSTSN (GAT x2 -> temporal MHA/FF -> cross MHA).

Sharding: 8 cores; core c owns graphs 4c..4c+3 of the B*S=32 graph axis
(= batch b=c//4, timesteps s=4*(c%4)..+4).  The reference's raw reshape
[B,S,N,32] -> [B*N, S, 32] makes each "temporal sequence" a contiguous
16-row block of the flat per-batch activation matrix, so the temporal
transformer stays local to the same shard.  Cross-batch exchange:
  - ReduceScatter (groups [[0..3],[4..7]]) for sf rows -> each core gets its
    own 256 query rows of sf = mean_s(spatial_out)
  - AllGather    (same groups)            for tf = mean_s2(t_out)  (K/V)
  - AllReduce    (all 8)                  for the final [1,2] output

GAT attention per (graph, head): logits src_i + dst_j are produced by a K=2
outer-sum matmul straight into PSUM; exp(leaky_relu(.)) is two ACT passes
(Prelu + Exp, both present in every PWP table set -> no table switches); the
softmax denominator comes free as row 0 of the alpha@[1|h] matmul (a ones
column prepended to the stationary operand).  All weights are repacked
host-side (block-diag GAT attention vectors, concatenated QKV, broadcast bias
tiles, mean factors folded into weights).
"""

import numpy as np

import concourse.bass as bass
import concourse.mybir as mybir
import concourse.tile as tile
from concourse.bass_utils import run_bass_kernel_spmd
from concourse.masks import make_identity
from concourse.vector_clock import ScopedClock

AF = mybir.ActivationFunctionType
ALU = mybir.AluOpType
FP = mybir.dt.float32
BF = mybir.dt.bfloat16

B, S, N, F_IN = 2, 16, 1024, 3
G_PER_CORE = 4
NC_CORES = 8
SEQ_PER_CORE = 256
ROWS = G_PER_CORE * N       # 4096 flat rows per core
NT = ROWS // 128            # 32 row-tiles
LN_EPS = 1e-3
ISQRT_KD = float(1.0 / np.sqrt(32.0))

# ---------------------------------------------------------------------------
# Workaround for this walrus build: codegen accepts at most ONE semaphore wait
# per instruction ("Too many sync wait commands"), but Tile attaches several.
# Excess waits ride on same-engine InstNoOp's placed immediately before (an
# engine drains its queue in order, so an earlier same-engine wait is
# equivalent); same treatment for the kernel-tail drain.
# ---------------------------------------------------------------------------
_uid = [0]


def _split_inst_waits(ordered):
    for bb_name, insts in ordered.items():
        new_list = []
        changed = False
        for inst in insts:
            si = getattr(inst, "sync_info", None)
            waits = list(si.on_wait) if si is not None and si.on_wait else []
            if len(waits) > 1:
                changed = True
                for w in waits[:-1]:
                    _uid[0] += 1
                    nop = mybir.InstNoOp(
                        name=f"I-wsplit-{_uid[0]}", engine=inst.engine, ins=[], outs=[]
                    )
                    nop.sync_info = mybir.SyncInfo(on_wait=[w], on_update=[])
                    nop.bass_nofuse = True
                    new_list.append(nop)
                si.on_wait = [waits[-1]]
            new_list.append(inst)
        if changed:
            ordered[bb_name] = new_list
    return ordered


_orig_lower = tile.TileContext._lower_ordered_insts


def _patched_lower(self, ordered):
    return _orig_lower(self, _split_inst_waits(ordered))


def _split_drain_and_barrier(self, tick_clock, wait_clock):
    nc = self.nc
    probe = nc.sync.nop(nofuse=True, hint="drain_wait_probe")
    wait_clock.add_sem_waits(probe.ins, ScopedClock({None: tick_clock.global_clock}))
    sync_info = probe.ins.sync_info
    waits = list(sync_info.on_wait) if sync_info is not None else []
    if len(waits) > 1:
        sync_info.on_wait = waits[:1]
        for w in waits[1:]:
            extra = nc.sync.nop(nofuse=True, hint="drain_wait_split")
            extra.ins.sync_info = mybir.SyncInfo(on_wait=[w], on_update=[])
    nc.sync.drain()
    nc.all_engine_barrier()
    assert self.sems is not None
    popped = nc._tile_sem_poison_stack.pop()
    assert popped is self._sem_poison
    nc.clear_and_free_semaphores(list(self.sems.allocated().values()))
    nc.all_engine_barrier()


def _install_patches():
    tile.TileContext._lower_ordered_insts = _patched_lower
    tile.TileContext._drain_and_barrier = _split_drain_and_barrier


# ---------------------------------------------------------------------------
# Host-side weight repacking
# ---------------------------------------------------------------------------


def _bcast(v, p=128):
    v = np.asarray(v, np.float32).reshape(1, -1)
    return np.ascontiguousarray(np.broadcast_to(v, (p, v.shape[1])))


def pack_consts(params):
    p = {k: np.asarray(v, np.float32) for k, v in params.items()}
    c = {}
    c["w1"] = p["gat1_w"]                                    # [3,128]
    a1 = np.zeros((128, 8), np.float32)
    for h in range(4):
        a1[32 * h : 32 * h + 32, h] = p["gat1_asrc"][h]
        a1[32 * h : 32 * h + 32, 4 + h] = p["gat1_adst"][h]
    c["A1"] = a1
    c["b1col"] = p["gat1_b"].reshape(128, 1)
    c["w2"] = p["gat2_w"]                                    # [128,32]
    a2 = np.zeros((32, 2), np.float32)
    a2[:, 0] = p["gat2_asrc"][0]
    a2[:, 1] = p["gat2_adst"][0]
    c["A2"] = a2
    c["b2col"] = p["gat2_b"].reshape(32, 1)

    c["wqkv"] = np.concatenate(
        [p["t_wq"].reshape(32, 64), p["t_wk"].reshape(32, 64), p["t_wv"].reshape(32, 64)], 1
    )
    c["bqkv_b"] = _bcast(
        np.concatenate([p["t_bq"].reshape(64), p["t_bk"].reshape(64), p["t_bv"].reshape(64)])
    )
    c["wo"] = p["t_wo"].reshape(64, 32)
    c["bo_b"] = _bcast(p["t_bo"])
    c["ln1g_b"] = _bcast(p["ln1_g"])
    c["ln1b_b"] = _bcast(p["ln1_b"])
    c["ln2g_b"] = _bcast(p["ln2_g"])
    c["ln2b_b"] = _bcast(p["ln2_b"])
    c["ffw1"] = p["ff_w1"]                                   # [32,64]
    c["ffb1_b"] = _bcast(p["ff_b1"])
    c["ffw2"] = p["ff_w2"]                                   # [64,32]
    c["ffb2_b"] = _bcast(p["ff_b2"])

    mask = np.zeros((128, 128), np.float32)
    for s8 in range(8):
        mask[16 * s8 : 16 * s8 + 16, 16 * s8 : 16 * s8 + 16] = 1.0
    c["mask_b"] = mask
    seg = np.zeros((128, 8), np.float32)
    for s8 in range(8):
        seg[16 * s8 : 16 * s8 + 16, s8] = 1.0 / 16.0         # tf mean folded
    c["seg"] = seg

    c["cwq_s"] = p["c_wq"].reshape(32, 64) / 16.0            # sf mean folded
    c["cbq_b"] = _bcast(p["c_bq"].reshape(64))
    c["cwk"] = p["c_wk"].reshape(32, 64)
    c["cbk_col"] = p["c_bk"].reshape(64, 1)
    c["cwv"] = p["c_wv"].reshape(32, 64)
    c["cbv_b"] = _bcast(p["c_bv"].reshape(64))
    c["cwo"] = p["c_wo"].reshape(64, 32)
    c["cbo_b"] = _bcast(p["c_bo"])
    c["fdw_s"] = p["fd_w"].reshape(32, 1) / 1024.0           # fused mean folded
    c["fdb"] = p["fd_b"].reshape(1, 1)
    c["ones_col"] = np.ones((128, 1), np.float32)
    c["ones_row"] = np.ones((1, 128), np.float32)
    c["ones_4N"] = np.ones((1, 4 * 1024), np.float32)
    c["eps_col"] = np.full((128, 1), LN_EPS, np.float32)
    return c


# ---------------------------------------------------------------------------
# Bass program (SPMD; identical program on all 8 cores)
# ---------------------------------------------------------------------------


def build_bass(const_shapes):
    _install_patches()
    nc = bass.Bass(num_devices=NC_CORES)

    xT = nc.dram_tensor("xT", [F_IN, ROWS], FP, kind="ExternalInput")
    onehot = nc.dram_tensor("onehot", [1, 2], FP, kind="ExternalInput")
    cin = {
        name: nc.dram_tensor(name, list(shape), FP, kind="ExternalInput")
        for name, shape in const_shapes.items()
    }
    out = nc.dram_tensor("out", [1, 2], FP, kind="ExternalOutput")

    GROUPS = [[0, 1, 2, 3], [4, 5, 6, 7]]
    ALL = [list(range(NC_CORES))]

    with tile.TileContext(nc) as tc:
        with (
            tc.tile_pool(name="const", bufs=1) as cp,
            tc.tile_pool(name="sb", bufs=2) as sb,
            tc.tile_pool(name="ebuf", bufs=3) as eb,
            tc.tile_pool(name="persist", bufs=1) as pp,
            tc.tile_pool(name="ps_big", bufs=2, space="PSUM") as ps_big,    # [128,1024] x2 = 4 banks
            tc.tile_pool(name="ps_tr", bufs=1, space="PSUM") as ps_tr,      # [128,128]       1 bank
            tc.tile_pool(name="ps_sm", bufs=1, space="PSUM") as ps_sm,      # [128,64]        1 bank
            tc.tile_pool(name="ps_acc", bufs=1, space="PSUM") as ps_acc,    # [33,1024]       2 banks
            tc.tile_pool(name="dram", bufs=1, space="DRAM") as dram,
        ):
            # ---- constants -------------------------------------------------
            C = {}
            for name, t in cin.items():
                C[name] = cp.tile(list(t.shape), FP, tag=f"c_{name}", name=f"c_{name}")
                nc.sync.dma_start(C[name][:], t[:])
            ident = cp.tile([128, 128], FP, tag="ident")
            make_identity(nc, ident[:])

            xTt = cp.tile([F_IN, ROWS], FP, tag="xT")
            nc.sync.dma_start(xTt[:], xT[:])
            oht = cp.tile([1, 2], FP, tag="onehot")
            nc.sync.dma_start(oht[:], onehot[:])

            def transpose_to(dst_sb_ap, src_sb_ap, pshape):
                # src [p,f] -> out [f,p]; identity must be [p,p]
                p = pshape[1]
                pt = ps_tr.tile([128, 128], FP, tag="tr")
                nc.tensor.transpose(pt[: pshape[0], : pshape[1]], src_sb_ap, ident[0:p, 0:p])
                nc.scalar.copy(dst_sb_ap, pt[: pshape[0], : pshape[1]])

            sfT = pp.tile([32, N], FP, tag="sfT")
            X = [pp.tile([128, 32], FP, tag=f"x{t}", name=f"x{t}") for t in range(NT)]

            # =========== Stage G: two GAT layers, per graph ================
            for g in range(G_PER_CORE):
                xg = xTt[:, g * N : (g + 1) * N]

                h1fP = ps_big.tile([128, N], FP, tag="bigP")
                for hf in range(2):
                    nc.tensor.matmul(
                        h1fP[:, 512 * hf : 512 * hf + 512],
                        C["w1"][:],
                        xg[:, 512 * hf : 512 * hf + 512],
                        start=True, stop=True,
                    )
                h1fT = sb.tile([128, N], FP, tag="h1fT", bufs=1)
                nc.scalar.copy(h1fT[:], h1fP[:])

                sdP = ps_big.tile([8, N], FP, tag="bigP")
                for hf in range(2):
                    nc.tensor.matmul(
                        sdP[:, 512 * hf : 512 * hf + 512],
                        C["A1"][:],
                        h1fT[:, 512 * hf : 512 * hf + 512],
                        start=True, stop=True,
                    )
                sd = sb.tile([8, N], FP, tag="sd", bufs=1)
                nc.scalar.copy(sd[:], sdP[:])
                # engine APs must start at partition 0/32/64, so the K=2
                # operand pairs are assembled with DMA (unrestricted):
                # L_all = [dst_h; ones], R_all = [ones; src_h] per head.
                L_all = sb.tile([2, 4, N], FP, tag="L_all", bufs=1)
                R_all = sb.tile([2, 4, N], FP, tag="R_all", bufs=1)
                nc.sync.dma_start(L_all[0:1, :, :], sd[4:8, :])
                nc.sync.dma_start(L_all[1:2, :, :], C["ones_4N"][:])
                nc.sync.dma_start(R_all[0:1, :, :], C["ones_4N"][:])
                nc.sync.dma_start(R_all[1:2, :, :], sd[0:4, :])

                # stationary [1|hfeat] tiles per j-chunk
                hw1 = []
                for j in range(8):
                    hwt = sb.tile([128, 132], BF, tag=f"hw1_{j}", bufs=1)
                    trP = ps_tr.tile([128, 128], FP, tag="tr")
                    nc.tensor.transpose(trP[:], h1fT[:, 128 * j : 128 * j + 128], ident[:])
                    for h in range(4):
                        nc.vector.tensor_copy(
                            hwt[:, 33 * h : 33 * h + 32],
                            trP[:, 32 * h : 32 * h + 32],
                        )
                    nc.gpsimd.memset(
                        hwt.rearrange("p (h c) -> p h c", c=33)[:, :, 32], 1.0
                    )
                    hw1.append(hwt)

                h1T = sb.tile([128, N], FP, tag="h1T", bufs=1)

                def gat_head(lhs_pair, rhs_pair, hw_tiles, out_cb):
                    """lhs_pair(j) -> [2,128] (dst_j, paired ones); rhs_pair ->
                    [2,N] (paired src).  out_cb(otP, rb): otP [33,N] PSUM accum
                    (row0 = softmax denom), rb [32,N] SBUF = 1/denom bcast."""
                    otP = ps_acc.tile([33, N], FP, tag="ot")
                    for j in range(8):
                        pP = ps_big.tile([128, N], FP, tag="bigP")
                        for hf in range(2):
                            nc.tensor.matmul(
                                pP[:, 512 * hf : 512 * hf + 512],
                                lhs_pair(j),
                                rhs_pair[:, 512 * hf : 512 * hf + 512],
                                start=True, stop=True,
                            )
                        lr = eb.tile([128, N], FP, tag="lrelu")
                        nc.scalar.activation(lr[:], pP[:], AF.Prelu, alpha=0.2)
                        et = eb.tile([128, N], BF, tag="et")
                        nc.scalar.activation(et[:], lr[:], AF.Exp)
                        for hf in range(2):
                            nc.tensor.matmul(
                                otP[:, 512 * hf : 512 * hf + 512],
                                hw_tiles[j],
                                et[:, 512 * hf : 512 * hf + 512],
                                start=(j == 0), stop=(j == 7),
                                skip_group_check=True,
                            )
                    rec = sb.tile([1, N], FP, tag="rec", bufs=1)
                    nc.vector.reciprocal(rec[:], otP[32:33, :])
                    rbP = ps_big.tile([32, N], FP, tag="bigP")
                    for hf in range(2):
                        nc.tensor.matmul(
                            rbP[:, 512 * hf : 512 * hf + 512],
                            C["ones_row"][:, 0:32],
                            rec[:, 512 * hf : 512 * hf + 512],
                            start=True, stop=True,
                        )
                    rb = sb.tile([32, N], FP, tag="rb", bufs=1)
                    nc.scalar.copy(rb[:], rbP[:])
                    out_cb(otP, rb)

                for h in range(4):
                    hw_h = [hw1[j][:, 33 * h : 33 * h + 33] for j in range(8)]
                    hN = sb.tile([32, N], FP, tag="hN", bufs=1)

                    def cb1(otP, rb, hN=hN):
                        nc.vector.tensor_tensor(
                            hN[:], otP[0:32, :], rb[:], op=ALU.mult
                        )

                    gat_head(
                        lambda j, h=h: L_all[:, h, 128 * j : 128 * j + 128],
                        R_all[:, h, :],
                        hw_h,
                        cb1,
                    )
                    nc.sync.dma_start(h1T[32 * h : 32 * h + 32, :], hN[:])
                nc.scalar.activation(h1T[:], h1T[:], AF.Relu, bias=C["b1col"][:])

                # ---- layer 2 ----
                h2fP = ps_big.tile([32, N], FP, tag="bigP")
                for hf in range(2):
                    nc.tensor.matmul(
                        h2fP[:, 512 * hf : 512 * hf + 512],
                        C["w2"][:],
                        h1T[:, 512 * hf : 512 * hf + 512],
                        start=True, stop=True,
                    )
                h2fT = sb.tile([32, N], FP, tag="h2fT", bufs=1)
                nc.scalar.copy(h2fT[:], h2fP[:])

                sd2P = ps_big.tile([2, N], FP, tag="bigP")
                for hf in range(2):
                    nc.tensor.matmul(
                        sd2P[:, 512 * hf : 512 * hf + 512],
                        C["A2"][:],
                        h2fT[:, 512 * hf : 512 * hf + 512],
                        start=True, stop=True,
                    )
                sd2 = sb.tile([2, N], FP, tag="sd2", bufs=1)
                nc.scalar.copy(sd2[:], sd2P[:])
                L2p = sb.tile([2, N], FP, tag="L2p", bufs=1)         # [dst; ones]
                R2p = sb.tile([2, N], FP, tag="R2p", bufs=1)         # [ones; src]
                nc.sync.dma_start(L2p[0:1, :], sd2[1:2, :])
                nc.sync.dma_start(L2p[1:2, :], C["ones_4N"][:, 0:N])
                nc.sync.dma_start(R2p[0:1, :], C["ones_4N"][:, 0:N])
                nc.sync.dma_start(R2p[1:2, :], sd2[0:1, :])

                hw2 = []
                for j in range(8):
                    hwt = sb.tile([128, 33], BF, tag=f"hw2_{j}", bufs=1)
                    trP = ps_tr.tile([128, 128], FP, tag="tr")
                    nc.tensor.transpose(trP[:, 0:32], h2fT[:, 128 * j : 128 * j + 128], ident[0:32, 0:32])
                    nc.vector.tensor_copy(hwt[:, 0:32], trP[:, 0:32])
                    nc.gpsimd.memset(hwt[:, 32:33], 1.0)
                    hw2.append(hwt)

                s2T = sb.tile([32, N], FP, tag="s2T", bufs=1)

                def cb2(otP, rb):
                    tmp = sb.tile([32, N], FP, tag="s2pre", bufs=1)
                    nc.vector.tensor_tensor(tmp[:], otP[0:32, :], rb[:], op=ALU.mult)
                    nc.scalar.activation(s2T[:], tmp[:], AF.Identity, bias=C["b2col"][:])

                gat_head(
                    lambda j: L2p[:, 128 * j : 128 * j + 128],
                    R2p[:],
                    hw2,
                    cb2,
                )

                if g == 0:
                    nc.vector.tensor_copy(sfT[:], s2T[:])
                else:
                    nc.vector.tensor_add(sfT[:], sfT[:], s2T[:])
                for j in range(8):
                    transpose_to(X[8 * g + j][:], s2T[:, 128 * j : 128 * j + 128], [128, 32])

            # =========== Stage T: temporal transformer =====================
            Y = [pp.tile([128, 32], FP, tag=f"y{t}", name=f"y{t}") for t in range(NT)]
            Gt = [pp.tile([128, 64], FP, tag=f"g{t}", name=f"g{t}") for t in range(NT)]
            tfTP = ps_acc.tile([32, SEQ_PER_CORE], FP, tag="ot")

            def layernorm(dst, src_sb, g_b, b_b):
                mu = sb.tile([128, 1], FP, tag="mu")
                nc.vector.reduce_sum(out=mu[:], in_=src_sb, axis=mybir.AxisListType.X)
                nc.scalar.mul(mu[:], mu[:], 1.0 / 32.0)
                xc = sb.tile([128, 32], FP, tag="xc")
                nc.vector.tensor_scalar(
                    out=xc[:], in0=src_sb, scalar1=mu[:], scalar2=None, op0=ALU.subtract
                )
                sq = sb.tile([128, 32], FP, tag="sq")
                ssq = sb.tile([128, 1], FP, tag="ssq")
                nc.scalar.activation(sq[:], xc[:], AF.Square, accum_out=ssq[:])
                lnv = sb.tile([128, 1], FP, tag="lnv")
                nc.scalar.activation(lnv[:], ssq[:], AF.Ln, bias=C["eps_col"][:], scale=1.0 / 32.0)
                rstd = sb.tile([128, 1], FP, tag="rstd")
                nc.scalar.activation(rstd[:], lnv[:], AF.Exp, scale=-0.5)
                nc.vector.tensor_scalar(
                    out=xc[:], in0=xc[:], scalar1=rstd[:], scalar2=None, op0=ALU.mult
                )
                nc.vector.tensor_tensor(xc[:], xc[:], g_b[:], op=ALU.mult)
                nc.vector.tensor_tensor(dst, xc[:], b_b[:], op=ALU.add)

            # --- T1: masked 8-seq MHA + LN1 (exp/ln tables only) ---
            for t in range(NT):
                xt_T = sb.tile([32, 128], FP, tag="xtT")
                transpose_to(xt_T[:], X[t][:], [32, 128])
                qkvP = ps_big.tile([128, 192], FP, tag="bigP")
                nc.tensor.matmul(qkvP[:], xt_T[:], C["wqkv"][:], start=True, stop=True)
                qkv = sb.tile([128, 192], FP, tag="qkv")
                nc.vector.tensor_tensor(qkv[:], qkvP[:], C["bqkv_b"][:], op=ALU.add)
                qT = sb.tile([64, 128], FP, tag="qT")
                kT = sb.tile([64, 128], FP, tag="kT")
                transpose_to(qT[:], qkv[:, 0:64], [64, 128])
                transpose_to(kT[:], qkv[:, 64:128], [64, 128])
                ot = sb.tile([128, 64], FP, tag="ot_t")
                for h in range(2):
                    lP = ps_big.tile([128, 128], FP, tag="bigP")
                    nc.tensor.matmul(
                        lP[:], qT[32 * h : 32 * h + 32, :], kT[32 * h : 32 * h + 32, :],
                        start=True, stop=True,
                    )
                    eT = sb.tile([128, 128], FP, tag="eT_t")
                    nc.scalar.activation(eT[:], lP[:], AF.Exp, scale=ISQRT_KD)
                    em = sb.tile([128, 128], FP, tag="em_t")
                    nc.vector.tensor_tensor(em[:], eT[:], C["mask_b"][:], op=ALU.mult)
                    rs = sb.tile([128, 1], FP, tag="rs_t")
                    nc.vector.reduce_sum(out=rs[:], in_=em[:], axis=mybir.AxisListType.X)
                    rc = sb.tile([128, 1], FP, tag="rc_t")
                    nc.vector.reciprocal(rc[:], rs[:])
                    aT = sb.tile([128, 128], FP, tag="aT_t")
                    transpose_to(aT[:], em[:], [128, 128])
                    oP = ps_sm.tile([128, 64], FP, tag="oP")
                    nc.tensor.matmul(
                        oP[:, 0:32], aT[:], qkv[:, 128 + 32 * h : 160 + 32 * h],
                        start=True, stop=True,
                    )
                    nc.vector.tensor_scalar(
                        out=ot[:, 32 * h : 32 * h + 32], in0=oP[:, 0:32], scalar1=rc[:],
                        scalar2=None, op0=ALU.mult,
                    )
                oT = sb.tile([64, 128], FP, tag="oT_t")
                transpose_to(oT[:], ot[:], [64, 128])
                atP = ps_sm.tile([128, 64], FP, tag="oP")
                nc.tensor.matmul(atP[:, 0:32], oT[:], C["wo"][:], start=True, stop=True)
                r1 = sb.tile([128, 32], FP, tag="r1")
                nc.vector.tensor_tensor(r1[:], atP[:, 0:32], C["bo_b"][:], op=ALU.add)
                nc.vector.tensor_tensor(r1[:], r1[:], X[t][:], op=ALU.add)
                layernorm(Y[t][:], r1[:], C["ln1g_b"], C["ln1b_b"])

            # --- T2: FF-in + gelu (gelu tables) ---
            for t in range(NT):
                yT = sb.tile([32, 128], FP, tag="yT")
                transpose_to(yT[:], Y[t][:], [32, 128])
                f1P = ps_sm.tile([128, 64], FP, tag="oP")
                nc.tensor.matmul(f1P[:], yT[:], C["ffw1"][:], start=True, stop=True)
                f1 = sb.tile([128, 64], FP, tag="f1")
                nc.vector.tensor_tensor(f1[:], f1P[:], C["ffb1_b"][:], op=ALU.add)
                nc.scalar.activation(Gt[t][:], f1[:], AF.Gelu)

            # --- T3: FF-out + LN2 + tf seg-reduce (exp/ln tables) ---
            for t in range(NT):
                gT = sb.tile([64, 128], FP, tag="gT")
                transpose_to(gT[:], Gt[t][:], [64, 128])
                f2P = ps_sm.tile([128, 64], FP, tag="oP")
                nc.tensor.matmul(f2P[:, 0:32], gT[:], C["ffw2"][:], start=True, stop=True)
                r2 = sb.tile([128, 32], FP, tag="r2")
                nc.vector.tensor_tensor(r2[:], f2P[:, 0:32], C["ffb2_b"][:], op=ALU.add)
                nc.vector.tensor_tensor(r2[:], r2[:], Y[t][:], op=ALU.add)
                y2 = sb.tile([128, 32], FP, tag="y2")
                layernorm(y2[:], r2[:], C["ln2g_b"], C["ln2b_b"])
                nc.tensor.matmul(
                    tfTP[:, 8 * t : 8 * t + 8], y2[:], C["seg"][:],
                    start=True, stop=True, skip_group_check=True,
                )

            tfT = pp.tile([32, SEQ_PER_CORE], FP, tag="tfT_sb")
            nc.scalar.copy(tfT[:], tfTP[:])

            # =========== Stage C: collectives ==============================
            # sf: row-layout partial, ReduceScatter -> my 256 query rows
            sf_rows = sb.tile([128, 8, 32], FP, tag="sf_rows", bufs=1)
            for j in range(8):
                transpose_to(sf_rows[:, j, :], sfT[:, 128 * j : 128 * j + 128], [128, 32])
            sf_in = dram.tile([N, 32], FP)
            sf_out = dram.tile([SEQ_PER_CORE, 32], FP)
            nc.sync.dma_start(sf_in[:].rearrange("(j p) d -> p j d", p=128), sf_rows[:])
            nc.gpsimd.collective_compute(
                "ReduceScatter", ALU.add, replica_groups=GROUPS,
                ins=[sf_in[:]], outs=[sf_out[:]],
            )
            sfmy = [pp.tile([128, 32], FP, tag=f"sfmy{k}", name=f"sfmy{k}") for k in range(2)]
            sfmyT = pp.tile([32, SEQ_PER_CORE], FP, tag="sfmyT")
            for k in range(2):
                nc.sync.dma_start(sfmy[k][:], sf_out[128 * k : 128 * k + 128, :])
                transpose_to(sfmyT[:, 128 * k : 128 * k + 128], sfmy[k][:], [32, 128])

            # tf: AllGather rows
            tf_rows = sb.tile([128, 2, 32], FP, tag="tf_rows", bufs=1)
            for k in range(2):
                transpose_to(tf_rows[:, k, :], tfT[:, 128 * k : 128 * k + 128], [128, 32])
            tf_in = dram.tile([SEQ_PER_CORE, 32], FP)
            tf_out = dram.tile([N, 32], FP)
            nc.sync.dma_start(tf_in[:].rearrange("(k p) d -> p k d", p=128), tf_rows[:])
            nc.gpsimd.collective_compute(
                "AllGather", ALU.bypass, replica_groups=GROUPS,
                ins=[tf_in[:]], outs=[tf_out[:]],
            )
            tfTf = pp.tile([32, N], FP, tag="tfTf")
            for j in range(8):
                tfr = sb.tile([128, 32], FP, tag="tfr")
                nc.sync.dma_start(tfr[:], tf_out[128 * j : 128 * j + 128, :])
                transpose_to(tfTf[:, 128 * j : 128 * j + 128], tfr[:], [32, 128])

            # =========== Stage X: cross attention (my 256 q rows) ==========
            ktP = ps_big.tile([64, N], FP, tag="bigP")
            for hf in range(2):
                nc.tensor.matmul(
                    ktP[:, 512 * hf : 512 * hf + 512],
                    C["cwk"][:],
                    tfTf[:, 512 * hf : 512 * hf + 512],
                    start=True, stop=True,
                )
            kt = sb.tile([64, N], FP, tag="kt", bufs=1)
            nc.scalar.activation(kt[:], ktP[:], AF.Identity, bias=C["cbk_col"][:])
            V = []
            for j in range(8):
                vP = ps_sm.tile([128, 64], FP, tag="oP")
                nc.tensor.matmul(
                    vP[:], tfTf[:, 128 * j : 128 * j + 128], C["cwv"][:],
                    start=True, stop=True,
                )
                vt = pp.tile([128, 64], FP, tag=f"v{j}")
                nc.vector.tensor_tensor(vt[:], vP[:], C["cbv_b"][:], op=ALU.add)
                V.append(vt)

            cs = sb.tile([32, 1], FP, tag="cs")
            nc.vector.memset(cs[:], 0.0)
            for st in range(2):
                qP = ps_sm.tile([128, 64], FP, tag="oP")
                nc.tensor.matmul(
                    qP[:], sfmyT[:, 128 * st : 128 * st + 128], C["cwq_s"][:],
                    start=True, stop=True,
                )
                q_t = sb.tile([128, 64], FP, tag="q_t")
                nc.vector.tensor_tensor(q_t[:], qP[:], C["cbq_b"][:], op=ALU.add)
                qT_t = sb.tile([64, 128], FP, tag="qT_t")
                transpose_to(qT_t[:], q_t[:], [64, 128])
                o_t = sb.tile([128, 64], FP, tag="o_t")
                for h in range(2):
                    lP = ps_big.tile([128, N], FP, tag="bigP")
                    for hf in range(2):
                        nc.tensor.matmul(
                            lP[:, 512 * hf : 512 * hf + 512],
                            qT_t[32 * h : 32 * h + 32, :],
                            kt[32 * h : 32 * h + 32, 512 * hf : 512 * hf + 512],
                            start=True, stop=True,
                        )
                    e_t = eb.tile([128, N], FP, tag="et")
                    rs = sb.tile([128, 1], FP, tag="rs_x")
                    nc.scalar.activation(
                        e_t[:], lP[:], AF.Exp, scale=ISQRT_KD, accum_out=rs[:]
                    )
                    rc = sb.tile([128, 1], FP, tag="rc_x")
                    nc.vector.reciprocal(rc[:], rs[:])
                    oaP = ps_acc.tile([128, 32], FP, tag="ot")
                    for j in range(8):
                        aT = sb.tile([128, 128], FP, tag="aT_x")
                        transpose_to(aT[:], e_t[:, 128 * j : 128 * j + 128], [128, 128])
                        nc.tensor.matmul(
                            oaP[:], aT[:], V[j][:, 32 * h : 32 * h + 32],
                            start=(j == 0), stop=(j == 7),
                            skip_group_check=True,
                        )
                    nc.vector.tensor_scalar(
                        out=o_t[:, 32 * h : 32 * h + 32], in0=oaP[:], scalar1=rc[:],
                        scalar2=None, op0=ALU.mult,
                    )
                oT_t = sb.tile([64, 128], FP, tag="oT_x")
                transpose_to(oT_t[:], o_t[:], [64, 128])
                fP = ps_sm.tile([128, 64], FP, tag="oP")
                nc.tensor.matmul(fP[:, 0:32], oT_t[:], C["cwo"][:], start=True, stop=True)
                fused_t = sb.tile([128, 32], FP, tag="fused")
                nc.vector.tensor_tensor(fused_t[:], fP[:, 0:32], C["cbo_b"][:], op=ALU.add)
                csP = ps_tr.tile([128, 128], FP, tag="tr")
                nc.tensor.matmul(
                    csP[0:32, 0:1], fused_t[:], C["ones_col"][:], start=True, stop=True
                )
                nc.vector.tensor_add(cs[:], cs[:], csP[0:32, 0:1])

            psP = ps_tr.tile([128, 128], FP, tag="tr")
            nc.tensor.matmul(psP[0:1, 0:1], cs[:], C["fdw_s"][:], start=True, stop=True)
            ps_sb = sb.tile([1, 1], FP, tag="ps_sb")
            nc.scalar.copy(ps_sb[:], psP[0:1, 0:1])
            ctP = ps_tr.tile([128, 128], FP, tag="tr")
            nc.tensor.matmul(ctP[0:1, 0:2], ps_sb[:], oht[:], start=True, stop=True)
            ct = sb.tile([1, 2], FP, tag="ct")
            nc.scalar.copy(ct[:], ctP[0:1, 0:2])

            ar_in = dram.tile([1, 2], FP)
            ar_out = dram.tile([1, 2], FP)
            nc.sync.dma_start(ar_in[:], ct[:])
            nc.gpsimd.collective_compute(
                "AllReduce", ALU.add, replica_groups=ALL,
                ins=[ar_in[:]], outs=[ar_out[:]],
            )
            fin = sb.tile([1, 2], FP, tag="fin")
            nc.sync.dma_start(fin[:], ar_out[:])
            res = sb.tile([1, 2], FP, tag="res")
            nc.scalar.activation(res[:], fin[:], AF.Identity, bias=C["fdb"][:])
            nc.sync.dma_start(out[:], res[:])

    return nc


# ---------------------------------------------------------------------------
# Public entry point
# ---------------------------------------------------------------------------

_CACHE = {}


def kernel(x, params):
    x = np.asarray(x, np.float32)
    assert x.shape == (B, S, N, F_IN), x.shape
    consts = pack_consts(params)
    const_shapes = {k: v.shape for k, v in consts.items()}

    if "nc" not in _CACHE:
        _CACHE["nc"] = build_bass(const_shapes)
    nc = _CACHE["nc"]

    in_maps = []
    for c in range(NC_CORES):
        b = c // 4
        xg = x[b, 4 * (c % 4) : 4 * (c % 4) + 4]          # [4,1024,3]
        xTv = np.ascontiguousarray(xg.reshape(ROWS, F_IN).T)
        onehot = np.zeros((1, 2), np.float32)
        onehot[0, b] = 1.0
        m = {"xT": xTv, "onehot": onehot}
        m.update(consts)
        in_maps.append(m)

    res = run_bass_kernel_spmd(nc, in_maps, list(range(NC_CORES))).results
    return np.asarray(res[0]["out"], np.float32).reshape(B, 1)


# revision 14
# speedup vs baseline: 275.5738x; 275.5738x over previous
"""Trainium2 Bass kernel for nn_EnhancedMSTSN (GAT x2 -> temporal MHA/FF -> cross MHA).

Sharding: 8 cores; core c owns graphs 4c..4c+3 of the B*S=32 graph axis
(= batch b=c//4, timesteps s=4*(c%4)..+4).  The reference's raw reshape
[B,S,N,32] -> [B*N, S, 32] makes each "temporal sequence" a contiguous
16-row block of the flat per-batch activation matrix, so the temporal
transformer stays local to the same shard.  Cross-batch exchange:
  - ReduceScatter (groups [[0..3],[4..7]]) for sf rows -> each core gets its
    own 256 query rows of sf = mean_s(spatial_out)
  - AllGather    (same groups)            for tf = mean_s2(t_out)  (K/V)
  - AllReduce    (all 8)                  for the final [1,2] output

GAT attention per (graph, head): logits src_i + dst_j are produced by a K=2
outer-sum matmul straight into PSUM; exp(leaky_relu(.)) is two ACT passes
(Prelu + Exp, both present in every PWP table set -> no table switches); the
softmax denominator comes free as row 0 of the alpha@[1|h] matmul (a ones
column prepended to the stationary operand).  All weights are repacked
host-side (block-diag GAT attention vectors, concatenated QKV, broadcast bias
tiles, mean factors folded into weights).
"""

import numpy as np

import concourse.bass as bass
import concourse.mybir as mybir
import concourse.tile as tile
from concourse.bass_utils import run_bass_kernel_spmd
from concourse.masks import make_identity
from concourse.vector_clock import ScopedClock

AF = mybir.ActivationFunctionType
ALU = mybir.AluOpType
FP = mybir.dt.float32
BF = mybir.dt.bfloat16

B, S, N, F_IN = 2, 16, 1024, 3
G_PER_CORE = 4
NC_CORES = 8
SEQ_PER_CORE = 256
ROWS = G_PER_CORE * N       # 4096 flat rows per core
NT = ROWS // 128            # 32 row-tiles
LN_EPS = 1e-3
ISQRT_KD = float(1.0 / np.sqrt(32.0))

# ---------------------------------------------------------------------------
# Workaround for this walrus build: codegen accepts at most ONE semaphore wait
# per instruction ("Too many sync wait commands"), but Tile attaches several.
# Excess waits ride on same-engine InstNoOp's placed immediately before (an
# engine drains its queue in order, so an earlier same-engine wait is
# equivalent); same treatment for the kernel-tail drain.
# ---------------------------------------------------------------------------
_uid = [0]


def _split_inst_waits(ordered):
    for bb_name, insts in ordered.items():
        new_list = []
        changed = False
        for inst in insts:
            si = getattr(inst, "sync_info", None)
            waits = list(si.on_wait) if si is not None and si.on_wait else []
            if len(waits) > 1:
                changed = True
                for w in waits[:-1]:
                    _uid[0] += 1
                    nop = mybir.InstNoOp(
                        name=f"I-wsplit-{_uid[0]}", engine=inst.engine, ins=[], outs=[]
                    )
                    nop.sync_info = mybir.SyncInfo(on_wait=[w], on_update=[])
                    nop.bass_nofuse = True
                    new_list.append(nop)
                si.on_wait = [waits[-1]]
            new_list.append(inst)
        if changed:
            ordered[bb_name] = new_list
    return ordered


_orig_lower = tile.TileContext._lower_ordered_insts


def _patched_lower(self, ordered):
    return _orig_lower(self, _split_inst_waits(ordered))


def _split_drain_and_barrier(self, tick_clock, wait_clock):
    nc = self.nc
    probe = nc.sync.nop(nofuse=True, hint="drain_wait_probe")
    wait_clock.add_sem_waits(probe.ins, ScopedClock({None: tick_clock.global_clock}))
    sync_info = probe.ins.sync_info
    waits = list(sync_info.on_wait) if sync_info is not None else []
    if len(waits) > 1:
        sync_info.on_wait = waits[:1]
        for w in waits[1:]:
            extra = nc.sync.nop(nofuse=True, hint="drain_wait_split")
            extra.ins.sync_info = mybir.SyncInfo(on_wait=[w], on_update=[])
    nc.sync.drain()
    nc.all_engine_barrier()
    assert self.sems is not None
    popped = nc._tile_sem_poison_stack.pop()
    assert popped is self._sem_poison
    nc.clear_and_free_semaphores(list(self.sems.allocated().values()))
    nc.all_engine_barrier()


def _install_patches():
    tile.TileContext._lower_ordered_insts = _patched_lower
    tile.TileContext._drain_and_barrier = _split_drain_and_barrier


# ---------------------------------------------------------------------------
# Host-side weight repacking
# ---------------------------------------------------------------------------


def _bcast(v, p=128):
    v = np.asarray(v, np.float32).reshape(1, -1)
    return np.ascontiguousarray(np.broadcast_to(v, (p, v.shape[1])))


def pack_consts(params):
    p = {k: np.asarray(v, np.float32) for k, v in params.items()}
    c = {}
    c["w1"] = p["gat1_w"]                                    # [3,128]
    a1 = np.zeros((128, 8), np.float32)
    for h in range(4):
        a1[32 * h : 32 * h + 32, h] = p["gat1_asrc"][h]
        a1[32 * h : 32 * h + 32, 4 + h] = p["gat1_adst"][h]
    c["A1"] = a1
    c["b1col"] = p["gat1_b"].reshape(128, 1)
    c["w2"] = p["gat2_w"]                                    # [128,32]
    a2 = np.zeros((32, 2), np.float32)
    a2[:, 0] = p["gat2_asrc"][0]
    a2[:, 1] = p["gat2_adst"][0]
    c["A2"] = a2
    c["b2col"] = p["gat2_b"].reshape(32, 1)

    c["wqkv"] = np.concatenate(
        [p["t_wq"].reshape(32, 64), p["t_wk"].reshape(32, 64), p["t_wv"].reshape(32, 64)], 1
    )
    c["bqkv_b"] = _bcast(
        np.concatenate([p["t_bq"].reshape(64), p["t_bk"].reshape(64), p["t_bv"].reshape(64)])
    )
    c["wo"] = p["t_wo"].reshape(64, 32)
    c["bo_b"] = _bcast(p["t_bo"])
    c["ln1g_b"] = _bcast(p["ln1_g"])
    c["ln1b_b"] = _bcast(p["ln1_b"])
    c["ln2g_b"] = _bcast(p["ln2_g"])
    c["ln2b_b"] = _bcast(p["ln2_b"])
    c["ffw1"] = p["ff_w1"]                                   # [32,64]
    c["ffb1_b"] = _bcast(p["ff_b1"])
    c["ffw2"] = p["ff_w2"]                                   # [64,32]
    c["ffb2_b"] = _bcast(p["ff_b2"])

    mask = np.zeros((128, 128), np.float32)
    for s8 in range(8):
        mask[16 * s8 : 16 * s8 + 16, 16 * s8 : 16 * s8 + 16] = 1.0
    c["mask_b"] = mask
    seg = np.zeros((128, 8), np.float32)
    for s8 in range(8):
        seg[16 * s8 : 16 * s8 + 16, s8] = 1.0 / 16.0         # tf mean folded
    c["seg"] = seg

    c["cwq_s"] = p["c_wq"].reshape(32, 64) / 16.0            # sf mean folded
    c["cbq_b"] = _bcast(p["c_bq"].reshape(64))
    c["cwk"] = p["c_wk"].reshape(32, 64)
    c["cbk_col"] = p["c_bk"].reshape(64, 1)
    c["cwv"] = p["c_wv"].reshape(32, 64)
    c["cbv_b"] = _bcast(p["c_bv"].reshape(64))
    c["cwo"] = p["c_wo"].reshape(64, 32)
    c["cbo_b"] = _bcast(p["c_bo"])
    c["fdw_s"] = p["fd_w"].reshape(32, 1) / 1024.0           # fused mean folded
    c["fdb"] = p["fd_b"].reshape(1, 1)
    c["ones_col"] = np.ones((128, 1), np.float32)
    c["ones_row"] = np.ones((1, 128), np.float32)
    c["ones_4N"] = np.ones((1, 4 * 1024), np.float32)
    c["eps_col"] = np.full((128, 1), LN_EPS, np.float32)
    return c


# ---------------------------------------------------------------------------
# Bass program (SPMD; identical program on all 8 cores)
# ---------------------------------------------------------------------------


def build_bass(const_shapes):
    _install_patches()
    nc = bass.Bass(num_devices=NC_CORES)

    xT = nc.dram_tensor("xT", [F_IN, ROWS], FP, kind="ExternalInput")
    onehot = nc.dram_tensor("onehot", [1, 2], FP, kind="ExternalInput")
    cin = {
        name: nc.dram_tensor(name, list(shape), FP, kind="ExternalInput")
        for name, shape in const_shapes.items()
    }
    out = nc.dram_tensor("out", [1, 2], FP, kind="ExternalOutput")

    GROUPS = [[0, 1, 2, 3], [4, 5, 6, 7]]
    ALL = [list(range(NC_CORES))]

    with tile.TileContext(nc) as tc:
        with (
            tc.tile_pool(name="const", bufs=1) as cp,
            tc.tile_pool(name="sb", bufs=2) as sb,
            tc.tile_pool(name="ebuf", bufs=3) as eb,
            tc.tile_pool(name="persist", bufs=1) as pp,
            tc.tile_pool(name="ps_big", bufs=2, space="PSUM") as ps_big,    # [128,1024] x2 = 4 banks
            tc.tile_pool(name="ps_tr", bufs=1, space="PSUM") as ps_tr,      # [128,128]       1 bank
            tc.tile_pool(name="ps_sm", bufs=1, space="PSUM") as ps_sm,      # [128,64]        1 bank
            tc.tile_pool(name="ps_acc", bufs=1, space="PSUM") as ps_acc,    # [33,1024]       2 banks
            tc.tile_pool(name="dram", bufs=1, space="DRAM") as dram,
        ):
            # ---- constants -------------------------------------------------
            C = {}
            for name, t in cin.items():
                C[name] = cp.tile(list(t.shape), FP, tag=f"c_{name}", name=f"c_{name}")
                nc.sync.dma_start(C[name][:], t[:])
            ident = cp.tile([128, 128], FP, tag="ident")
            make_identity(nc, ident[:])

            xTt = cp.tile([F_IN, ROWS], FP, tag="xT")
            nc.sync.dma_start(xTt[:], xT[:])
            oht = cp.tile([1, 2], FP, tag="onehot")
            nc.sync.dma_start(oht[:], onehot[:])

            def transpose_to(dst_sb_ap, src_sb_ap, pshape):
                # src [p,f] -> out [f,p]; identity must be [p,p]
                p = pshape[1]
                pt = ps_tr.tile([128, 128], FP, tag="tr")
                nc.tensor.transpose(pt[: pshape[0], : pshape[1]], src_sb_ap, ident[0:p, 0:p])
                nc.scalar.copy(dst_sb_ap, pt[: pshape[0], : pshape[1]])

            sfT = pp.tile([32, N], FP, tag="sfT")
            X = [pp.tile([128, 32], FP, tag=f"x{t}", name=f"x{t}") for t in range(NT)]

            # =========== Stage G: two GAT layers, per graph ================
            for g in range(G_PER_CORE):
                xg = xTt[:, g * N : (g + 1) * N]

                h1fP = ps_big.tile([128, N], FP, tag="bigP")
                for hf in range(2):
                    nc.tensor.matmul(
                        h1fP[:, 512 * hf : 512 * hf + 512],
                        C["w1"][:],
                        xg[:, 512 * hf : 512 * hf + 512],
                        start=True, stop=True,
                    )
                h1fT = sb.tile([128, N], FP, tag="h1fT", bufs=1)
                nc.scalar.copy(h1fT[:], h1fP[:])

                sdP = ps_big.tile([8, N], FP, tag="bigP")
                for hf in range(2):
                    nc.tensor.matmul(
                        sdP[:, 512 * hf : 512 * hf + 512],
                        C["A1"][:],
                        h1fT[:, 512 * hf : 512 * hf + 512],
                        start=True, stop=True,
                    )
                sd = sb.tile([8, N], FP, tag="sd", bufs=1)
                nc.scalar.copy(sd[:], sdP[:])
                # engine APs must start at partition 0/32/64, so the K=2
                # operand pairs are assembled with DMA (unrestricted):
                # L_all = [dst_h; ones], R_all = [ones; src_h] per head.
                L_all = sb.tile([2, 4, N], FP, tag="L_all", bufs=1)
                R_all = sb.tile([2, 4, N], FP, tag="R_all", bufs=1)
                nc.sync.dma_start(L_all[0:1, :, :], sd[4:8, :])
                nc.sync.dma_start(L_all[1:2, :, :], C["ones_4N"][:])
                nc.sync.dma_start(R_all[0:1, :, :], C["ones_4N"][:])
                nc.sync.dma_start(R_all[1:2, :, :], sd[0:4, :])

                # stationary [1|hfeat] tiles per j-chunk
                hw1 = []
                for j in range(8):
                    hwt = sb.tile([128, 132], BF, tag=f"hw1_{j}", bufs=1)
                    trP = ps_tr.tile([128, 128], FP, tag="tr")
                    nc.tensor.transpose(trP[:], h1fT[:, 128 * j : 128 * j + 128], ident[:])
                    for h in range(4):
                        nc.vector.tensor_copy(
                            hwt[:, 33 * h : 33 * h + 32],
                            trP[:, 32 * h : 32 * h + 32],
                        )
                    nc.gpsimd.memset(
                        hwt.rearrange("p (h c) -> p h c", c=33)[:, :, 32], 1.0
                    )
                    hw1.append(hwt)

                h1T = sb.tile([128, N], FP, tag="h1T", bufs=1)

                def gat_head(lhs_pair, rhs_pair, hw_tiles, out_cb):
                    """lhs_pair(j) -> [2,128] (dst_j, paired ones); rhs_pair ->
                    [2,N] (paired src).  out_cb(otP, rb): otP [33,N] PSUM accum
                    (row0 = softmax denom), rb [32,N] SBUF = 1/denom bcast."""
                    otP = ps_acc.tile([33, N], FP, tag="ot")
                    for j in range(8):
                        pP = ps_big.tile([128, N], FP, tag="bigP")
                        for hf in range(2):
                            nc.tensor.matmul(
                                pP[:, 512 * hf : 512 * hf + 512],
                                lhs_pair(j),
                                rhs_pair[:, 512 * hf : 512 * hf + 512],
                                start=True, stop=True,
                            )
                        lr = eb.tile([128, N], FP, tag="lrelu", bufs=2)
                        nc.scalar.activation(lr[:], pP[:], AF.Prelu, alpha=0.2)
                        et = eb.tile([128, N], BF, tag="et", bufs=4)
                        nc.scalar.activation(et[:], lr[:], AF.Exp)
                        for hf in range(2):
                            nc.tensor.matmul(
                                otP[:, 512 * hf : 512 * hf + 512],
                                hw_tiles[j],
                                et[:, 512 * hf : 512 * hf + 512],
                                start=(j == 0), stop=(j == 7),
                                skip_group_check=True,
                            )
                    otS = sb.tile([33, N], FP, tag="otS", bufs=1)
                    nc.scalar.copy(otS[:], otP[:])
                    rec = sb.tile([1, N], FP, tag="rec", bufs=1)
                    nc.vector.reciprocal(rec[:], otS[32:33, :])
                    rbP = ps_big.tile([32, N], FP, tag="bigP")
                    for hf in range(2):
                        nc.tensor.matmul(
                            rbP[:, 512 * hf : 512 * hf + 512],
                            C["ones_row"][:, 0:32],
                            rec[:, 512 * hf : 512 * hf + 512],
                            start=True, stop=True,
                        )
                    rb = sb.tile([32, N], FP, tag="rb", bufs=1)
                    nc.scalar.copy(rb[:], rbP[:])
                    out_cb(otS, rb)

                for h in range(4):
                    hw_h = [hw1[j][:, 33 * h : 33 * h + 33] for j in range(8)]
                    hN = sb.tile([32, N], FP, tag="hN", bufs=1)

                    def cb1(otP, rb, hN=hN):
                        nc.vector.tensor_tensor(
                            hN[:], otP[0:32, :], rb[:], op=ALU.mult
                        )

                    gat_head(
                        lambda j, h=h: L_all[:, h, 128 * j : 128 * j + 128],
                        R_all[:, h, :],
                        hw_h,
                        cb1,
                    )
                    nc.sync.dma_start(h1T[32 * h : 32 * h + 32, :], hN[:])
                nc.scalar.activation(h1T[:], h1T[:], AF.Relu, bias=C["b1col"][:])

                # ---- layer 2 ----
                h2fP = ps_big.tile([32, N], FP, tag="bigP")
                for hf in range(2):
                    nc.tensor.matmul(
                        h2fP[:, 512 * hf : 512 * hf + 512],
                        C["w2"][:],
                        h1T[:, 512 * hf : 512 * hf + 512],
                        start=True, stop=True,
                    )
                h2fT = sb.tile([32, N], FP, tag="h2fT", bufs=1)
                nc.scalar.copy(h2fT[:], h2fP[:])

                sd2P = ps_big.tile([2, N], FP, tag="bigP")
                for hf in range(2):
                    nc.tensor.matmul(
                        sd2P[:, 512 * hf : 512 * hf + 512],
                        C["A2"][:],
                        h2fT[:, 512 * hf : 512 * hf + 512],
                        start=True, stop=True,
                    )
                sd2 = sb.tile([2, N], FP, tag="sd2", bufs=1)
                nc.scalar.copy(sd2[:], sd2P[:])
                L2p = sb.tile([2, N], FP, tag="L2p", bufs=1)         # [dst; ones]
                R2p = sb.tile([2, N], FP, tag="R2p", bufs=1)         # [ones; src]
                nc.sync.dma_start(L2p[0:1, :], sd2[1:2, :])
                nc.sync.dma_start(L2p[1:2, :], C["ones_4N"][:, 0:N])
                nc.sync.dma_start(R2p[0:1, :], C["ones_4N"][:, 0:N])
                nc.sync.dma_start(R2p[1:2, :], sd2[0:1, :])

                hw2 = []
                for j in range(8):
                    hwt = sb.tile([128, 33], BF, tag=f"hw2_{j}", bufs=1)
                    trP = ps_tr.tile([128, 128], FP, tag="tr")
                    nc.tensor.transpose(trP[:, 0:32], h2fT[:, 128 * j : 128 * j + 128], ident[0:32, 0:32])
                    nc.vector.tensor_copy(hwt[:, 0:32], trP[:, 0:32])
                    nc.gpsimd.memset(hwt[:, 32:33], 1.0)
                    hw2.append(hwt)

                s2T = sb.tile([32, N], FP, tag="s2T", bufs=1)

                def cb2(otP, rb):
                    tmp = sb.tile([32, N], FP, tag="s2pre", bufs=1)
                    nc.vector.tensor_tensor(tmp[:], otP[0:32, :], rb[:], op=ALU.mult)
                    nc.scalar.activation(s2T[:], tmp[:], AF.Identity, bias=C["b2col"][:])

                gat_head(
                    lambda j: L2p[:, 128 * j : 128 * j + 128],
                    R2p[:],
                    hw2,
                    cb2,
                )

                if g == 0:
                    nc.vector.tensor_copy(sfT[:], s2T[:])
                else:
                    nc.vector.tensor_add(sfT[:], sfT[:], s2T[:])
                for j in range(8):
                    transpose_to(X[8 * g + j][:], s2T[:, 128 * j : 128 * j + 128], [128, 32])

            # =========== Stage T: temporal transformer =====================
            Y = [pp.tile([128, 32], FP, tag=f"y{t}", name=f"y{t}") for t in range(NT)]
            Gt = [pp.tile([128, 64], FP, tag=f"g{t}", name=f"g{t}") for t in range(NT)]
            tfTP = ps_acc.tile([32, SEQ_PER_CORE], FP, tag="ot")

            def layernorm(dst, src_sb, g_b, b_b):
                mu = sb.tile([128, 1], FP, tag="mu")
                nc.vector.reduce_sum(out=mu[:], in_=src_sb, axis=mybir.AxisListType.X)
                nc.scalar.mul(mu[:], mu[:], 1.0 / 32.0)
                xc = sb.tile([128, 32], FP, tag="xc")
                nc.vector.tensor_scalar(
                    out=xc[:], in0=src_sb, scalar1=mu[:], scalar2=None, op0=ALU.subtract
                )
                sq = sb.tile([128, 32], FP, tag="sq")
                ssq = sb.tile([128, 1], FP, tag="ssq")
                nc.scalar.activation(sq[:], xc[:], AF.Square, accum_out=ssq[:])
                lnv = sb.tile([128, 1], FP, tag="lnv")
                nc.scalar.activation(lnv[:], ssq[:], AF.Ln, bias=C["eps_col"][:], scale=1.0 / 32.0)
                rstd = sb.tile([128, 1], FP, tag="rstd")
                nc.scalar.activation(rstd[:], lnv[:], AF.Exp, scale=-0.5)
                nc.vector.tensor_scalar(
                    out=xc[:], in0=xc[:], scalar1=rstd[:], scalar2=None, op0=ALU.mult
                )
                nc.vector.tensor_tensor(xc[:], xc[:], g_b[:], op=ALU.mult)
                nc.vector.tensor_tensor(dst, xc[:], b_b[:], op=ALU.add)

            # --- T1: masked 8-seq MHA + LN1 (exp/ln tables only) ---
            for t in range(NT):
                xt_T = sb.tile([32, 128], FP, tag="xtT")
                transpose_to(xt_T[:], X[t][:], [32, 128])
                qkvP = ps_big.tile([128, 192], FP, tag="bigP")
                nc.tensor.matmul(qkvP[:], xt_T[:], C["wqkv"][:], start=True, stop=True)
                qkv = sb.tile([128, 192], FP, tag="qkv")
                nc.vector.tensor_tensor(qkv[:], qkvP[:], C["bqkv_b"][:], op=ALU.add)
                qT = sb.tile([64, 128], FP, tag="qT")
                kT = sb.tile([64, 128], FP, tag="kT")
                transpose_to(qT[:], qkv[:, 0:64], [64, 128])
                transpose_to(kT[:], qkv[:, 64:128], [64, 128])
                ot = sb.tile([128, 64], FP, tag="ot_t")
                for h in range(2):
                    lP = ps_big.tile([128, 128], FP, tag="bigP")
                    nc.tensor.matmul(
                        lP[:], qT[32 * h : 32 * h + 32, :], kT[32 * h : 32 * h + 32, :],
                        start=True, stop=True,
                    )
                    eT = sb.tile([128, 128], FP, tag="eT_t")
                    nc.scalar.activation(eT[:], lP[:], AF.Exp, scale=ISQRT_KD)
                    em = sb.tile([128, 128], FP, tag="em_t")
                    nc.vector.tensor_tensor(em[:], eT[:], C["mask_b"][:], op=ALU.mult)
                    rs = sb.tile([128, 1], FP, tag="rs_t")
                    nc.vector.reduce_sum(out=rs[:], in_=em[:], axis=mybir.AxisListType.X)
                    rc = sb.tile([128, 1], FP, tag="rc_t")
                    nc.vector.reciprocal(rc[:], rs[:])
                    aT = sb.tile([128, 128], FP, tag="aT_t")
                    transpose_to(aT[:], em[:], [128, 128])
                    oP = ps_sm.tile([128, 64], FP, tag="oP")
                    nc.tensor.matmul(
                        oP[:, 0:32], aT[:], qkv[:, 128 + 32 * h : 160 + 32 * h],
                        start=True, stop=True,
                    )
                    nc.vector.tensor_scalar(
                        out=ot[:, 32 * h : 32 * h + 32], in0=oP[:, 0:32], scalar1=rc[:],
                        scalar2=None, op0=ALU.mult,
                    )
                oT = sb.tile([64, 128], FP, tag="oT_t")
                transpose_to(oT[:], ot[:], [64, 128])
                atP = ps_sm.tile([128, 64], FP, tag="oP")
                nc.tensor.matmul(atP[:, 0:32], oT[:], C["wo"][:], start=True, stop=True)
                r1 = sb.tile([128, 32], FP, tag="r1")
                nc.vector.tensor_tensor(r1[:], atP[:, 0:32], C["bo_b"][:], op=ALU.add)
                nc.vector.tensor_tensor(r1[:], r1[:], X[t][:], op=ALU.add)
                layernorm(Y[t][:], r1[:], C["ln1g_b"], C["ln1b_b"])

            # --- T2: FF-in + gelu (gelu tables) ---
            for t in range(NT):
                yT = sb.tile([32, 128], FP, tag="yT")
                transpose_to(yT[:], Y[t][:], [32, 128])
                f1P = ps_sm.tile([128, 64], FP, tag="oP")
                nc.tensor.matmul(f1P[:], yT[:], C["ffw1"][:], start=True, stop=True)
                f1 = sb.tile([128, 64], FP, tag="f1")
                nc.vector.tensor_tensor(f1[:], f1P[:], C["ffb1_b"][:], op=ALU.add)
                nc.scalar.activation(Gt[t][:], f1[:], AF.Gelu)

            # --- T3: FF-out + LN2 + tf seg-reduce (exp/ln tables) ---
            for t in range(NT):
                gT = sb.tile([64, 128], FP, tag="gT")
                transpose_to(gT[:], Gt[t][:], [64, 128])
                f2P = ps_sm.tile([128, 64], FP, tag="oP")
                nc.tensor.matmul(f2P[:, 0:32], gT[:], C["ffw2"][:], start=True, stop=True)
                r2 = sb.tile([128, 32], FP, tag="r2")
                nc.vector.tensor_tensor(r2[:], f2P[:, 0:32], C["ffb2_b"][:], op=ALU.add)
                nc.vector.tensor_tensor(r2[:], r2[:], Y[t][:], op=ALU.add)
                y2 = sb.tile([128, 32], FP, tag="y2")
                layernorm(y2[:], r2[:], C["ln2g_b"], C["ln2b_b"])
                nc.tensor.matmul(
                    tfTP[:, 8 * t : 8 * t + 8], y2[:], C["seg"][:],
                    start=True, stop=True, skip_group_check=True,
                )

            tfT = pp.tile([32, SEQ_PER_CORE], FP, tag="tfT_sb")
            nc.scalar.copy(tfT[:], tfTP[:])

            # =========== Stage C: collectives ==============================
            # sf: row-layout partial, ReduceScatter -> my 256 query rows
            sf_rows = sb.tile([128, 8, 32], FP, tag="sf_rows", bufs=1)
            for j in range(8):
                transpose_to(sf_rows[:, j, :], sfT[:, 128 * j : 128 * j + 128], [128, 32])
            sf_in = dram.tile([N, 32], FP)
            sf_out = dram.tile([SEQ_PER_CORE, 32], FP)
            nc.sync.dma_start(sf_in[:].rearrange("(j p) d -> p j d", p=128), sf_rows[:])
            nc.gpsimd.collective_compute(
                "ReduceScatter", ALU.add, replica_groups=GROUPS,
                ins=[sf_in[:]], outs=[sf_out[:]],
            )
            sfmy = [pp.tile([128, 32], FP, tag=f"sfmy{k}", name=f"sfmy{k}") for k in range(2)]
            sfmyT = pp.tile([32, SEQ_PER_CORE], FP, tag="sfmyT")
            for k in range(2):
                nc.sync.dma_start(sfmy[k][:], sf_out[128 * k : 128 * k + 128, :])
                transpose_to(sfmyT[:, 128 * k : 128 * k + 128], sfmy[k][:], [32, 128])

            # tf: AllGather rows
            tf_rows = sb.tile([128, 2, 32], FP, tag="tf_rows", bufs=1)
            for k in range(2):
                transpose_to(tf_rows[:, k, :], tfT[:, 128 * k : 128 * k + 128], [128, 32])
            tf_in = dram.tile([SEQ_PER_CORE, 32], FP)
            tf_out = dram.tile([N, 32], FP)
            nc.sync.dma_start(tf_in[:].rearrange("(k p) d -> p k d", p=128), tf_rows[:])
            nc.gpsimd.collective_compute(
                "AllGather", ALU.bypass, replica_groups=GROUPS,
                ins=[tf_in[:]], outs=[tf_out[:]],
            )
            tfTf = pp.tile([32, N], FP, tag="tfTf")
            for j in range(8):
                tfr = sb.tile([128, 32], FP, tag="tfr")
                nc.sync.dma_start(tfr[:], tf_out[128 * j : 128 * j + 128, :])
                transpose_to(tfTf[:, 128 * j : 128 * j + 128], tfr[:], [32, 128])

            # =========== Stage X: cross attention (my 256 q rows) ==========
            ktP = ps_big.tile([64, N], FP, tag="bigP")
            for hf in range(2):
                nc.tensor.matmul(
                    ktP[:, 512 * hf : 512 * hf + 512],
                    C["cwk"][:],
                    tfTf[:, 512 * hf : 512 * hf + 512],
                    start=True, stop=True,
                )
            kt = sb.tile([64, N], FP, tag="kt", bufs=1)
            nc.scalar.activation(kt[:], ktP[:], AF.Identity, bias=C["cbk_col"][:])
            V = []
            for j in range(8):
                vP = ps_sm.tile([128, 64], FP, tag="oP")
                nc.tensor.matmul(
                    vP[:], tfTf[:, 128 * j : 128 * j + 128], C["cwv"][:],
                    start=True, stop=True,
                )
                vt = pp.tile([128, 64], FP, tag=f"v{j}")
                nc.vector.tensor_tensor(vt[:], vP[:], C["cbv_b"][:], op=ALU.add)
                V.append(vt)

            cs = sb.tile([32, 1], FP, tag="cs")
            nc.vector.memset(cs[:], 0.0)
            for st in range(2):
                qP = ps_sm.tile([128, 64], FP, tag="oP")
                nc.tensor.matmul(
                    qP[:], sfmyT[:, 128 * st : 128 * st + 128], C["cwq_s"][:],
                    start=True, stop=True,
                )
                q_t = sb.tile([128, 64], FP, tag="q_t")
                nc.vector.tensor_tensor(q_t[:], qP[:], C["cbq_b"][:], op=ALU.add)
                qT_t = sb.tile([64, 128], FP, tag="qT_t")
                transpose_to(qT_t[:], q_t[:], [64, 128])
                o_t = sb.tile([128, 64], FP, tag="o_t")
                for h in range(2):
                    lP = ps_big.tile([128, N], FP, tag="bigP")
                    for hf in range(2):
                        nc.tensor.matmul(
                            lP[:, 512 * hf : 512 * hf + 512],
                            qT_t[32 * h : 32 * h + 32, :],
                            kt[32 * h : 32 * h + 32, 512 * hf : 512 * hf + 512],
                            start=True, stop=True,
                        )
                    e_t = eb.tile([128, N], FP, tag="et_x", bufs=2)
                    rs = sb.tile([128, 1], FP, tag="rs_x")
                    nc.scalar.activation(
                        e_t[:], lP[:], AF.Exp, scale=ISQRT_KD, accum_out=rs[:]
                    )
                    rc = sb.tile([128, 1], FP, tag="rc_x")
                    nc.vector.reciprocal(rc[:], rs[:])
                    oaP = ps_acc.tile([128, 32], FP, tag="ot")
                    for j in range(8):
                        aT = sb.tile([128, 128], FP, tag="aT_x")
                        transpose_to(aT[:], e_t[:, 128 * j : 128 * j + 128], [128, 128])
                        nc.tensor.matmul(
                            oaP[:], aT[:], V[j][:, 32 * h : 32 * h + 32],
                            start=(j == 0), stop=(j == 7),
                            skip_group_check=True,
                        )
                    nc.vector.tensor_scalar(
                        out=o_t[:, 32 * h : 32 * h + 32], in0=oaP[:], scalar1=rc[:],
                        scalar2=None, op0=ALU.mult,
                    )
                oT_t = sb.tile([64, 128], FP, tag="oT_x")
                transpose_to(oT_t[:], o_t[:], [64, 128])
                fP = ps_sm.tile([128, 64], FP, tag="oP")
                nc.tensor.matmul(fP[:, 0:32], oT_t[:], C["cwo"][:], start=True, stop=True)
                fused_t = sb.tile([128, 32], FP, tag="fused")
                nc.vector.tensor_tensor(fused_t[:], fP[:, 0:32], C["cbo_b"][:], op=ALU.add)
                csP = ps_tr.tile([128, 128], FP, tag="tr")
                nc.tensor.matmul(
                    csP[0:32, 0:1], fused_t[:], C["ones_col"][:], start=True, stop=True
                )
                nc.vector.tensor_add(cs[:], cs[:], csP[0:32, 0:1])

            psP = ps_tr.tile([128, 128], FP, tag="tr")
            nc.tensor.matmul(psP[0:1, 0:1], cs[:], C["fdw_s"][:], start=True, stop=True)
            ps_sb = sb.tile([1, 1], FP, tag="ps_sb")
            nc.scalar.copy(ps_sb[:], psP[0:1, 0:1])
            ctP = ps_tr.tile([128, 128], FP, tag="tr")
            nc.tensor.matmul(ctP[0:1, 0:2], ps_sb[:], oht[:], start=True, stop=True)
            ct = sb.tile([1, 2], FP, tag="ct")
            nc.scalar.copy(ct[:], ctP[0:1, 0:2])

            ar_in = dram.tile([1, 2], FP)
            ar_out = dram.tile([1, 2], FP)
            nc.sync.dma_start(ar_in[:], ct[:])
            nc.gpsimd.collective_compute(
                "AllReduce", ALU.add, replica_groups=ALL,
                ins=[ar_in[:]], outs=[ar_out[:]],
            )
            fin = sb.tile([1, 2], FP, tag="fin")
            nc.sync.dma_start(fin[:], ar_out[:])
            res = sb.tile([1, 2], FP, tag="res")
            nc.scalar.activation(res[:], fin[:], AF.Identity, bias=C["fdb"][:])
            nc.sync.dma_start(out[:], res[:])

    return nc


# ---------------------------------------------------------------------------
# Public entry point
# ---------------------------------------------------------------------------

_CACHE = {}


def kernel(x, params):
    x = np.asarray(x, np.float32)
    assert x.shape == (B, S, N, F_IN), x.shape
    consts = pack_consts(params)
    const_shapes = {k: v.shape for k, v in consts.items()}

    if "nc" not in _CACHE:
        _CACHE["nc"] = build_bass(const_shapes)
    nc = _CACHE["nc"]

    in_maps = []
    for c in range(NC_CORES):
        b = c // 4
        xg = x[b, 4 * (c % 4) : 4 * (c % 4) + 4]          # [4,1024,3]
        xTv = np.ascontiguousarray(xg.reshape(ROWS, F_IN).T)
        onehot = np.zeros((1, 2), np.float32)
        onehot[0, b] = 1.0
        m = {"xT": xTv, "onehot": onehot}
        m.update(consts)
        in_maps.append(m)

    res = run_bass_kernel_spmd(nc, in_maps, list(range(NC_CORES))).results
    return np.asarray(res[0]["out"], np.float32).reshape(B, 1)
